# revision 27
# baseline (speedup 1.0000x reference)
"""Trainium2 Bass kernel for single-head full-dim attention (nn_CasualSelfAttention).

Reference math (B=4, S=4096, D=768, fp32):
    q = x @ Wq.T + bq ; k = x @ Wk.T + bk ; v = x @ Wv.T + bv
    att = softmax(q @ k.T * D**-0.5)        # no mask
    y = att @ v
    y = y.transpose(0,2,1).reshape(B,S,D)   # element permutation
    out = y @ Wc.T + bc

Sharding (8 cores): core c = 2*b + h handles batch b with ALL 4096 queries but
only its half of the keys/values (rows h*2048:(h+1)*2048). Each core produces a
partial unnormalized yT [768, 4096] (features x queries) plus partial softmax
sums, with the value bias folded in linearly (bv x partial_sums). A pairwise
ReduceScatter(add) in bf16, chunked per 512-query block, hands core h the
fully-reduced feature slice [384*h : 384*h+384] for all queries — exactly the
rows of y.T that the permutation maps to output rows [2048*h : 2048*h+2048].
After normalizing by the (also-reduced) sums, the flat buffer IS y_perm
row-major, and the final projection runs locally.

Perf notes vs the first working version:
  - Host pre-transposes x (xqT/xkvT) so no on-device DMA transposes are needed
    for the projections; biases ship as one packed [128,18] tensor (the 18
    separate 512B DMAs used to serialize ~12us of sync-engine time at startup).
  - The per-qc epilogue (sums reduce matmul in bf16, bias-fold, yTaug stores,
    RS issue) is emitted at kt==0 of the FOLLOWING qc, and the attention
    y-matmuls run at pipeline depth 2 (AV(kt-2) issues at kt), so the tensor
    engine has queued work while the vector engine drains the boundary chain.
  - Softmax-sum normalization of block b runs mid-loop (kt==10) of block b+1,
    using reciprocal_approx_fast, with the bf16->fp32 sum copy on the scalar
    engine — keeps the vector FIFO clear at qc boundaries.
  - Phase F uses 4 [512,768] DMA transposes instead of 16 [128,768] ones
    (the sync engine serializes ~2.4us per transpose issue).
"""

import numpy as np
import ml_dtypes

BF16 = ml_dtypes.bfloat16

B, S, D = 4, 4096, 768
SK = S // 2            # keys per core
P = 128
DT = D // P            # 6 feature tiles
KT = SK // P           # 16 key tiles
QC = 512               # query chunk width
NQC = S // QC          # 8 query chunks (= RS blocks)
FH = D // 2            # 384: feature rows per RS chunk
SCALE = float(D) ** -0.5
GROUPS = [[0, 1], [2, 3], [4, 5], [6, 7]]
AVD = 2                # AV pipeline depth: AV(kt-AVD) issues at kt

_nc = None


def _build_program():
    import concourse.bass as bass
    import concourse.mybir as mybir
    import concourse.tile as tile
    from concourse import bacc

    f32 = mybir.dt.float32
    bf16 = mybir.dt.bfloat16
    f8 = mybir.dt.float8e4
    DR = mybir.MatmulPerfMode.DoubleRow
    Exp = mybir.ActivationFunctionType.Exp
    Copy = mybir.ActivationFunctionType.Copy
    mult = mybir.AluOpType.mult
    add = mybir.AluOpType.add

    nc = bacc.Bacc(None, num_devices=8)

    xqT = nc.declare_dram_parameter("xqT", [D, S], bf16, isOutput=False)
    xkvT = nc.declare_dram_parameter("xkvT", [D, SK], bf16, isOutput=False)
    wqT = nc.declare_dram_parameter("wqT", [D, D], bf16, isOutput=False)
    wkT = nc.declare_dram_parameter("wkT", [D, D], bf16, isOutput=False)
    wvT = nc.declare_dram_parameter("wvT", [D, D], bf16, isOutput=False)
    wcT = nc.declare_dram_parameter("wcT", [D, D], bf16, isOutput=False)
    bqkv = nc.declare_dram_parameter("bqkv", [P, 3 * DT], f32, isOutput=False)
    bc = nc.declare_dram_parameter("bc", [1, D], f32, isOutput=False)
    out = nc.declare_dram_parameter("out", [SK, D], f32, isOutput=True)

    def wload(dst, src, split=True):
        # [768, 768] row-major -> [128, 6, 768] with logical row g*128+p.
        # split=True issues 6 independent per-g DMAs (parallel queues, lands
        # ~6x sooner); split=False is one strided DMA for non-urgent loads.
        if split:
            for g in range(DT):
                nc.sync.dma_start(dst[:, g, :], src[g * P:(g + 1) * P, :])
        else:
            nc.sync.dma_start(dst[:], src[:].rearrange("(g p) d -> p g d", p=P))

    with tile.TileContext(nc) as tc:
        with tc.tile_pool(name="persist", bufs=1) as pp, \
             tc.tile_pool(name="dram", bufs=1, space="DRAM") as dram:
            # Per block: rows 0:384 = feats 0:384, row 384 = partial sums,
            # rows 385:769 = feats 384:768, row 769 = partial sums.
            yTaug = [dram.tile([2 * (FH + 1), QC], bf16, name=f"yTaug{b}", tag=f"yTaug{b}")
                     for b in range(NQC)]
            rs_out = [dram.tile([FH + 1, QC], bf16, name=f"rs_out{b}", tag=f"rs_out{b}")
                      for b in range(NQC)]
            f_dram = dram.tile([SK, D], bf16)
            warm_cc_in = dram.tile([D, QC], bf16, name="warm_cc_in", tag="warm_cc_in")
            warm_cc_out = dram.tile([D // 2, QC], bf16, name="warm_cc_out", tag="warm_cc_out")

            # persistent SBUF: kT/qT (fp8, 3D so DoubleRow can pair feature
            # subtiles) + v activations + biases + output weights
            kT8 = pp.tile([P, DT, SK], f8, name="kT8", tag="kT8")
            qT8 = pp.tile([P, DT, S], f8, name="qT8", tag="qT8")
            v_sb = [pp.tile([P, D], bf16, name=f"v{t}", tag=f"v{t}") for t in range(KT)]
            bqkv_sb = pp.tile([P, 3 * DT], f32, tag="bqkv_sb")
            ones_sb = pp.tile([P, P], bf16, name="ones", tag="ones")
            wc_sb = pp.tile([P, DT, D], bf16, tag="wc_sb")
            bc_sb = pp.tile([1, D], f32, tag="bc_sb")
            bcb = pp.tile([P, D], f32, tag="bcb")

            def bQ(g):
                return bqkv_sb[:, g:g + 1]

            def bK(g):
                return bqkv_sb[:, DT + g:DT + g + 1]

            def bV(g):
                return bqkv_sb[:, 2 * DT + g:2 * DT + g + 1]

            # ---- Phase A: kT [768, 2048] and v [2048, 768] from xkvT ----
            import contextlib
            _ab_stack = contextlib.ExitStack()
            pa = _ab_stack.enter_context(tc.tile_pool(name="pA", bufs=1))
            with tc.tile_pool(name="psA", bufs=2, space="PSUM") as psa:
                wk_sb = pa.tile([P, DT, D], bf16, tag="wk_sb")
                wload(wk_sb, wkT)
                xkT0 = pa.tile([P, DT, QC], bf16, tag="xkT", bufs=2, name="xkT")
                for g in range(DT):
                    nc.sync.dma_start(xkT0[:, g, :], xkvT[g * P:(g + 1) * P, 0:QC])
                nc.sync.dma_start(bqkv_sb[:], bqkv[:])
                wv_sb = pa.tile([P, DT, D], bf16, tag="wv_sb")
                wload(wv_sb, wvT)
                # Warm the PE while the first loads land: keeps the engine busy
                # through the HAM activity window so the projections start at
                # the full 2.4 GHz clock instead of the throttled 1.2 GHz.
                nc.vector.memset(ones_sb[:], 1.0)
                warm_ps = psa.tile([P, P], f32, tag="warm", bufs=1)
                for _ in range(65):
                    nc.tensor.matmul(warm_ps[:], ones_sb[:], ones_sb[:],
                                     start=True, stop=True)
                # Warm the collectives path too: the first collective of a run
                # pays ~20us of extra ncfw/ring setup — absorb it here, where
                # it overlaps the projections, instead of on block 0's RS.
                # Full-size so the descriptor rings are staged for the real
                # transfer size.
                nc.sync.dma_start(warm_cc_in[:], xkvT[:, 0:QC])
                nc.gpsimd.collective_compute(
                    "ReduceScatter", mybir.AluOpType.add,
                    replica_groups=GROUPS,
                    ins=[warm_cc_in.opt()], outs=[warm_cc_out.opt()])
                for c in range(SK // QC):
                    if c == 0:
                        xkT = xkT0
                    else:
                        xkT = pa.tile([P, DT, QC], bf16, tag="xkT", bufs=2, name="xkT")
                        for g in range(DT):
                            nc.sync.dma_start(
                                xkT[:, g, :], xkvT[g * P:(g + 1) * P, c * QC:(c + 1) * QC])
                    for go in range(DT):
                        ps = psa.tile([P, QC], f32, tag="pk")
                        for gi in range(DT):
                            nc.tensor.matmul(
                                ps[:], wk_sb[:, gi, go * P:(go + 1) * P],
                                xkT[:, gi, :],
                                start=(gi == 0), stop=(gi == DT - 1))
                        nc.vector.tensor_scalar_add(
                            kT8[:, go, c * QC:(c + 1) * QC], ps[:], bK(go))
                    for tl in range(QC // P):
                        t = c * 4 + tl
                        for half in range(2):
                            ps = psa.tile([P, FH], f32, tag="pv", bufs=4)
                            for gi in range(DT):
                                nc.tensor.matmul(
                                    ps[:], xkT[:, gi, tl * P:(tl + 1) * P],
                                    wv_sb[:, gi, half * FH:(half + 1) * FH],
                                    start=(gi == 0), stop=(gi == DT - 1))
                            nc.vector.tensor_copy(v_sb[t][:, half * FH:(half + 1) * FH], ps[:])

            # ---- Phase B: qT [768, 4096] from xqT ----
            with tc.tile_pool(name="psB", bufs=2, space="PSUM") as psb:
                pb = pa
                wq_sb = pb.tile([P, DT, D], bf16, tag="wq_sb")
                wload(wq_sb, wqT)
                for c in range(NQC):
                    xqTt = pb.tile([P, DT, QC], bf16, tag="xqTt", bufs=4, name="xqTt")
                    for g in range(DT):
                        nc.sync.dma_start(
                            xqTt[:, g, :], xqT[g * P:(g + 1) * P, c * QC:(c + 1) * QC])
                    for go in range(DT):
                        ps = psb.tile([P, QC], f32, tag="pq")
                        for gi in range(DT):
                            nc.tensor.matmul(
                                ps[:], wq_sb[:, gi, go * P:(go + 1) * P],
                                xqTt[:, gi, :],
                                start=(gi == 0), stop=(gi == DT - 1))
                        nc.vector.tensor_scalar_add(
                            qT8[:, go, c * QC:(c + 1) * QC], ps[:], bQ(go))
            _ab_stack.close()

            # ---- Phase C: attention; per-qc deferred epilogue; chunked RS ----
            # Loads needed later emitted here so they overlap attention.
            wload(wc_sb, wcT, split=False)
            nc.sync.dma_start(bc_sb[:], bc[:])
            nc.gpsimd.partition_broadcast(bcb[:], bc_sb[:])

            with tc.tile_pool(name="pC", bufs=2) as pc, \
                 tc.tile_pool(name="pE", bufs=2) as pe, \
                 tc.tile_pool(name="psC", bufs=1, space="PSUM") as psc:
                f_view = f_dram[:].rearrange("a b -> (a b)").rearrange(
                    "(x c) -> x c", c=S)

                def emit_norm(b):
                    s_row = pe.tile([1, QC], bf16, tag="s_row", name="s_row")
                    nc.sync.dma_start(s_row[:], rs_out[b][FH:FH + 1, :])
                    frs = []
                    for r in range(FH // P):
                        fr = pe.tile([P, QC], bf16, tag="fr", bufs=3, name="fr")
                        nc.sync.dma_start(fr[:], rs_out[b][r * P:(r + 1) * P, :])
                        frs.append(fr)
                    s32 = pe.tile([1, QC], f32, tag="s32", name="s32")
                    nc.scalar.activation(s32[:], s_row[:], Copy)
                    rec = pe.tile([1, QC], f32, tag="rec", name="rec")
                    nc.vector.reciprocal_approx_fast(rec[:], s32[:])
                    rbc = pe.tile([P, QC], f32, tag="rbc", name="rbc")
                    nc.gpsimd.partition_broadcast(rbc[:], rec[:])
                    for r in range(FH // P):
                        fn = pe.tile([P, QC], bf16, tag="fn", bufs=2, name="fn")
                        nc.vector.tensor_mul(fn[:], frs[r][:], rbc[:])
                        nc.sync.dma_start(
                            f_view[r * P:(r + 1) * P, b * QC:(b + 1) * QC], fn[:])

                pend = {"epi": None, "norms": []}

                def emit_epilogue():
                    # deferred store half of the qc epilogue: bias-fold the
                    # finished ypsum into bf16 yTaug, then kick the RS.
                    blk, sbc, ypsum = pend["epi"]
                    pend["epi"] = None
                    yb = yTaug[blk]
                    nc.sync.dma_start(yb[FH:FH + 1, :], sbc[0:1, :])
                    nc.sync.dma_start(yb[2 * FH + 1:2 * FH + 2, :], sbc[0:1, :])
                    for e in range(DT):
                        yt_sb = pc.tile([P, QC], bf16, tag="yt_sb", bufs=3)
                        # (sums_bcast * bv[e]) + ypsum  — folds the value bias
                        nc.vector.scalar_tensor_tensor(
                            yt_sb[:], sbc[:], bV(e), ypsum[e][:], mult, add)
                        row = e * P if e < 3 else (FH + 1) + (e - 3) * P
                        nc.sync.dma_start(yb[row:row + P, :], yt_sb[:])
                    nc.gpsimd.collective_compute(
                        "ReduceScatter", mybir.AluOpType.add,
                        replica_groups=GROUPS,
                        ins=[yTaug[blk].opt()], outs=[rs_out[blk].opt()])
                    pend["norms"].append(blk)

                for qc in range(NQC):
                    sums_acc = pc.tile([P, QC], f32, tag="sums_acc")
                    nc.vector.memset(sums_acc[:], 0.0)
                    ypsum = None
                    a_tiles = {}
                    for kt in range(KT):
                        aps = psc.tile([P, QC], f32, tag="att", bufs=2)
                        for g in range(DT // 2):
                            # fp8 DoubleRow: each matmul contracts TWO 128-row
                            # feature subtiles (lhsT/rhs are [128, 2, free])
                            nc.tensor.matmul(
                                aps[:], kT8[:, 2 * g:2 * g + 2, kt * P:(kt + 1) * P],
                                qT8[:, 2 * g:2 * g + 2, qc * QC:(qc + 1) * QC],
                                start=(g == 0), stop=(g == DT // 2 - 1),
                                perf_mode=DR)
                        if kt == 0:
                            # previous qc's store half lands here: the vector
                            # chain it gates (stt x6) drains while this qc's QK
                            # matmuls keep the tensor engine busy.
                            if pend["epi"] is not None:
                                emit_epilogue()
                            ypsum = [psc.tile([P, QC], f32, name=f"y{e}", tag=f"y{e}", bufs=1)
                                     for e in range(DT)]
                        if kt >= AVD:
                            for e in range(DT):
                                nc.tensor.matmul(
                                    ypsum[e][:], v_sb[kt - AVD][:, e * P:(e + 1) * P],
                                    a_tiles[kt - AVD][:],
                                    start=(kt - AVD == 0), stop=False)
                        if kt == 4:
                            # normalize blocks whose RS was issued >= 3 qc ago
                            # (~96us; the RS pipeline incl. backlog is ~45us
                            # deep) — mid-loop, where the vector queue has
                            # slack. The last few blocks drain in the tail.
                            while pend["norms"] and pend["norms"][0] <= qc - 3:
                                emit_norm(pend["norms"].pop(0))
                        a_sb = pc.tile([P, QC], bf16, tag="a_sb", bufs=4)
                        a_tiles[kt] = a_sb
                        nc.scalar.activation(a_sb[:], aps[:], Exp, scale=SCALE)
                        nc.vector.tensor_add(sums_acc[:], sums_acc[:], a_sb[:])
                    sbc = None
                    for j, kt in enumerate(range(KT - AVD, KT)):
                        for e in range(DT):
                            nc.tensor.matmul(
                                ypsum[e][:], v_sb[kt][:, e * P:(e + 1) * P],
                                a_tiles[kt][:],
                                start=(kt == 0), stop=(kt == KT - 1))
                        if kt == KT - 2:
                            # sums half of the epilogue: cross-partition sum +
                            # broadcast via bf16 ones matmul, slotted between
                            # the AV tail groups so the cast has tensor cover.
                            sums_bf = pc.tile([P, QC], bf16, tag="sums_bf")
                            nc.vector.tensor_copy(sums_bf[:], sums_acc[:])
                            sp = psc.tile([P, QC], f32, tag="att", bufs=2)
                            nc.tensor.matmul(sp[:], ones_sb[:], sums_bf[:],
                                             start=True, stop=True)
                            sbc = pc.tile([P, QC], bf16, tag="sbc")
                            nc.vector.tensor_copy(sbc[:], sp[:])
                    pend["epi"] = (qc, sbc, ypsum)

                emit_epilogue()
                while pend["norms"]:
                    emit_norm(pend["norms"].pop(0))

            # ---- Phase F: out = y_perm @ Wc.T + bc ----
            with tc.tile_pool(name="pF", bufs=1) as pf, \
                 tc.tile_pool(name="psF", bufs=2, space="PSUM") as psf:
                fT4s = []
                for T in range(SK // QC):
                    fT4 = pf.tile([P, DT, QC], bf16, name=f"fT4_{T}", tag=f"fT4_{T}")
                    nc.sync.dma_start_transpose(fT4[:], f_dram[T * QC:(T + 1) * QC, :])
                    fT4s.append(fT4)
                for t in range(SK // P):
                    fT = fT4s[t // 4]
                    r0 = (t % 4) * P
                    po = psf.tile([P, QC], f32, tag="po", bufs=3)
                    po2 = psf.tile([P, D - QC], f32, tag="po2", bufs=3)
                    for gi in range(DT):
                        nc.tensor.matmul(po[:], fT[:, gi, r0:r0 + P], wc_sb[:, gi, 0:QC],
                                         start=(gi == 0), stop=(gi == DT - 1))
                        nc.tensor.matmul(po2[:], fT[:, gi, r0:r0 + P], wc_sb[:, gi, QC:D],
                                         start=(gi == 0), stop=(gi == DT - 1))
                    o_sb = pf.tile([P, D], f32, tag="o_sb", bufs=3)
                    nc.vector.tensor_add(o_sb[:, 0:QC], po[:], bcb[:, 0:QC])
                    nc.vector.tensor_add(o_sb[:, QC:D], po2[:], bcb[:, QC:D])
                    nc.sync.dma_start(out[t * P:(t + 1) * P, :], o_sb[:])

    return nc


def _get_nc():
    global _nc
    if _nc is None:
        _nc = _build_program()
        _nc.finalize()
    return _nc


def _prep_in_maps(x, Wq, bq, Wk, bk, Wv, bv, Wc, bc):
    x = np.asarray(x, dtype=np.float32)
    wqT = np.ascontiguousarray(np.asarray(Wq, np.float32).T).astype(BF16)
    wkT = np.ascontiguousarray(np.asarray(Wk, np.float32).T).astype(BF16)
    wvT = np.ascontiguousarray(np.asarray(Wv, np.float32).T).astype(BF16)
    wcT = np.ascontiguousarray(np.asarray(Wc, np.float32).T).astype(BF16)
    bqkv = np.concatenate(
        [np.asarray(b, np.float32).reshape(DT, P).T for b in (bq, bk, bv)],
        axis=1).copy()
    bcc = np.asarray(bc, np.float32).reshape(1, D).copy()
    xTs = [np.ascontiguousarray(x[b].T).astype(BF16) for b in range(B)]
    in_maps = []
    for c in range(8):
        b, h = divmod(c, 2)
        in_maps.append({
            "xqT": xTs[b],
            "xkvT": np.ascontiguousarray(xTs[b][:, h * SK:(h + 1) * SK]),
            "wqT": wqT, "wkT": wkT, "wvT": wvT, "wcT": wcT,
            "bqkv": bqkv, "bc": bcc,
        })
    return in_maps


def _assemble(results):
    out = np.empty((B, S, D), dtype=np.float32)
    for c in range(8):
        b, h = divmod(c, 2)
        out[b, h * SK:(h + 1) * SK, :] = results[c]["out"]
    return out


def run_on_hw(trace=False, **inputs):
    from concourse.bass_utils import run_bass_kernel_spmd
    nc = _get_nc()
    in_maps = _prep_in_maps(**inputs)
    res = run_bass_kernel_spmd(nc, in_maps, list(range(8)), trace=trace)
    return _assemble(res.results), res


def kernel(**inputs):
    out, _ = run_on_hw(trace=False, **inputs)
    return out


# revision 34
# speedup vs baseline: 1.0837x; 1.0837x over previous
"""Trainium2 Bass kernel for single-head full-dim attention (nn_CasualSelfAttention).

Reference math (B=4, S=4096, D=768, fp32):
    q = x @ Wq.T + bq ; k = x @ Wk.T + bk ; v = x @ Wv.T + bv
    att = softmax(q @ k.T * D**-0.5)        # no mask
    y = att @ v
    y = y.transpose(0,2,1).reshape(B,S,D)   # element permutation
    out = y @ Wc.T + bc

Sharding (8 cores): core c = 2*b + h handles batch b with ALL 4096 queries but
only its half of the keys/values (rows h*2048:(h+1)*2048). Each core produces a
partial unnormalized yT [768, 4096] (features x queries) plus partial softmax
sums, with the value bias folded in linearly (bv x partial_sums). A pairwise
ReduceScatter(add) in bf16, chunked per 512-query block, hands core h the
fully-reduced feature slice [384*h : 384*h+384] for all queries — exactly the
rows of y.T that the permutation maps to output rows [2048*h : 2048*h+2048].
After normalizing by the (also-reduced) sums, the flat buffer IS y_perm
row-major, and the final projection runs locally.

Perf notes vs the first working version:
  - Host pre-transposes x (xqT/xkvT) so no on-device DMA transposes are needed
    for the projections; biases ship as one packed [128,18] tensor (the 18
    separate 512B DMAs used to serialize ~12us of sync-engine time at startup).
  - The per-qc epilogue (sums reduce matmul in bf16, bias-fold, yTaug stores,
    RS issue) is emitted at kt==0 of the FOLLOWING qc, and the attention
    y-matmuls run at pipeline depth 2 (AV(kt-2) issues at kt), so the tensor
    engine has queued work while the vector engine drains the boundary chain.
  - Softmax-sum normalization of block b runs mid-loop (kt==10) of block b+1,
    using reciprocal_approx_fast, with the bf16->fp32 sum copy on the scalar
    engine — keeps the vector FIFO clear at qc boundaries.
  - Phase F uses 4 [512,768] DMA transposes instead of 16 [128,768] ones
    (the sync engine serializes ~2.4us per transpose issue).
"""

import numpy as np
import ml_dtypes

BF16 = ml_dtypes.bfloat16

B, S, D = 4, 4096, 768
SK = S // 2            # keys per core
P = 128
DT = D // P            # 6 feature tiles
KT = SK // P           # 16 key tiles
QC = 512               # query chunk width
NQC = S // QC          # 8 query chunks (= RS blocks)
FH = D // 2            # 384: feature rows per RS chunk
SCALE = float(D) ** -0.5
GROUPS = [[0, 1], [2, 3], [4, 5], [6, 7]]
AVD = 2                # AV pipeline depth: AV(kt-AVD) issues at kt

_nc = None


def _build_program():
    import concourse.bass as bass
    import concourse.mybir as mybir
    import concourse.tile as tile
    from concourse import bacc

    f32 = mybir.dt.float32
    bf16 = mybir.dt.bfloat16
    f8 = mybir.dt.float8e4
    DR = mybir.MatmulPerfMode.DoubleRow
    Exp = mybir.ActivationFunctionType.Exp
    Copy = mybir.ActivationFunctionType.Copy
    mult = mybir.AluOpType.mult
    add = mybir.AluOpType.add

    nc = bacc.Bacc(None, num_devices=8)

    xq8 = nc.declare_dram_parameter("xq8", [D, S], f8, isOutput=False)
    xkv8 = nc.declare_dram_parameter("xkv8", [D, SK], f8, isOutput=False)
    xkvT = nc.declare_dram_parameter("xkvT", [D, SK], bf16, isOutput=False)
    wq8T = nc.declare_dram_parameter("wq8T", [D, D], f8, isOutput=False)
    wk8T = nc.declare_dram_parameter("wk8T", [D, D], f8, isOutput=False)
    wvT = nc.declare_dram_parameter("wvT", [D, D], bf16, isOutput=False)
    wcT = nc.declare_dram_parameter("wcT", [D, D], bf16, isOutput=False)
    bqkv = nc.declare_dram_parameter("bqkv", [P, 3 * DT], f32, isOutput=False)
    bc = nc.declare_dram_parameter("bc", [1, D], f32, isOutput=False)
    out = nc.declare_dram_parameter("out", [SK, D], f32, isOutput=True)

    def wload(dst, src, split=True):
        # [768, 768] row-major -> [128, 6, 768] with logical row g*128+p.
        # split=True issues 6 independent per-g DMAs (parallel queues, lands
        # ~6x sooner); split=False is one strided DMA for non-urgent loads.
        if split:
            for g in range(DT):
                nc.sync.dma_start(dst[:, g, :], src[g * P:(g + 1) * P, :])
        else:
            nc.sync.dma_start(dst[:], src[:].rearrange("(g p) d -> p g d", p=P))

    with tile.TileContext(nc) as tc:
        with tc.tile_pool(name="persist", bufs=1) as pp, \
             tc.tile_pool(name="dram", bufs=1, space="DRAM") as dram:
            # Per block: rows 0:384 = feats 0:384, row 384 = partial sums,
            # rows 385:769 = feats 384:768, row 769 = partial sums.
            yTaug = [dram.tile([2 * (FH + 1), QC], bf16, name=f"yTaug{b}", tag=f"yTaug{b}")
                     for b in range(NQC)]
            rs_out = [dram.tile([FH + 1, QC], bf16, name=f"rs_out{b}", tag=f"rs_out{b}")
                      for b in range(NQC)]
            f_dram = dram.tile([SK, D], bf16)
            warm_cc_in = dram.tile([D, QC], bf16, name="warm_cc_in", tag="warm_cc_in")
            warm_cc_out = dram.tile([D // 2, QC], bf16, name="warm_cc_out", tag="warm_cc_out")

            # persistent SBUF: kT/qT (fp8, 3D so DoubleRow can pair feature
            # subtiles) + v activations + biases + output weights
            kT8 = pp.tile([P, DT, SK], f8, name="kT8", tag="kT8")
            qT8 = pp.tile([P, DT, S], f8, name="qT8", tag="qT8")
            v_sb = [pp.tile([P, D], bf16, name=f"v{t}", tag=f"v{t}") for t in range(KT)]
            bqkv_sb = pp.tile([P, 3 * DT], f32, tag="bqkv_sb")
            ones_sb = pp.tile([P, P], bf16, name="ones", tag="ones")
            wc_sb = pp.tile([P, DT, D], bf16, tag="wc_sb")
            bc_sb = pp.tile([1, D], f32, tag="bc_sb")
            bcb = pp.tile([P, D], f32, tag="bcb")

            def bQ(g):
                return bqkv_sb[:, g:g + 1]

            def bK(g):
                return bqkv_sb[:, DT + g:DT + g + 1]

            def bV(g):
                return bqkv_sb[:, 2 * DT + g:2 * DT + g + 1]

            # ---- Phase A: kT [768, 2048] and v [2048, 768] from xkvT ----
            import contextlib
            _ab_stack = contextlib.ExitStack()
            pa = _ab_stack.enter_context(tc.tile_pool(name="pA", bufs=1))
            with tc.tile_pool(name="psA", bufs=2, space="PSUM") as psa:
                wk_sb = pa.tile([P, DT, D], f8, tag="wk_sb")
                wload(wk_sb, wk8T)
                xk80 = pa.tile([P, DT, QC], f8, tag="xk8", bufs=2, name="xk8")
                for g in range(DT):
                    nc.sync.dma_start(xk80[:, g, :], xkv8[g * P:(g + 1) * P, 0:QC])
                nc.sync.dma_start(bqkv_sb[:], bqkv[:])
                xkT0 = pa.tile([P, DT, QC], bf16, tag="xkT", bufs=2, name="xkT")
                for g in range(DT):
                    nc.sync.dma_start(xkT0[:, g, :], xkvT[g * P:(g + 1) * P, 0:QC])
                wv_sb = pa.tile([P, DT, D], bf16, tag="wv_sb")
                wload(wv_sb, wvT)
                # Warm the PE while the first loads land: keeps the engine busy
                # through the HAM activity window so the projections start at
                # the full 2.4 GHz clock instead of the throttled 1.2 GHz.
                nc.vector.memset(ones_sb[:], 1.0)
                warm_ps = psa.tile([P, P], f32, tag="warm", bufs=1)
                for _ in range(65):
                    nc.tensor.matmul(warm_ps[:], ones_sb[:], ones_sb[:],
                                     start=True, stop=True)
                # Warm the collectives path too: the first collective of a run
                # pays ~20us of extra ncfw/ring setup — absorb it here, where
                # it overlaps the projections, instead of on block 0's RS.
                # Full-size so the descriptor rings are staged for the real
                # transfer size.
                nc.sync.dma_start(warm_cc_in[:], xkvT[:, 0:QC])
                nc.gpsimd.collective_compute(
                    "ReduceScatter", mybir.AluOpType.add,
                    replica_groups=GROUPS,
                    ins=[warm_cc_in.opt()], outs=[warm_cc_out.opt()])
                for c in range(SK // QC):
                    if c == 0:
                        xk8, xkT = xk80, xkT0
                    else:
                        xk8 = pa.tile([P, DT, QC], f8, tag="xk8", bufs=2, name="xk8")
                        for g in range(DT):
                            nc.sync.dma_start(
                                xk8[:, g, :], xkv8[g * P:(g + 1) * P, c * QC:(c + 1) * QC])
                        xkT = pa.tile([P, DT, QC], bf16, tag="xkT", bufs=2, name="xkT")
                        for g in range(DT):
                            nc.sync.dma_start(
                                xkT[:, g, :], xkvT[g * P:(g + 1) * P, c * QC:(c + 1) * QC])
                    for go in range(DT):
                        ps = psa.tile([P, QC], f32, tag="pk")
                        for g in range(DT // 2):
                            nc.tensor.matmul(
                                ps[:], wk_sb[:, 2 * g:2 * g + 2, go * P:(go + 1) * P],
                                xk8[:, 2 * g:2 * g + 2, :],
                                start=(g == 0), stop=(g == DT // 2 - 1),
                                perf_mode=DR)
                        nc.vector.tensor_scalar_add(
                            kT8[:, go, c * QC:(c + 1) * QC], ps[:], bK(go))
                    for tl in range(QC // P):
                        t = c * 4 + tl
                        for half in range(2):
                            ps = psa.tile([P, FH], f32, tag="pv", bufs=4)
                            for gi in range(DT):
                                nc.tensor.matmul(
                                    ps[:], xkT[:, gi, tl * P:(tl + 1) * P],
                                    wv_sb[:, gi, half * FH:(half + 1) * FH],
                                    start=(gi == 0), stop=(gi == DT - 1))
                            nc.vector.tensor_copy(v_sb[t][:, half * FH:(half + 1) * FH], ps[:])

            # ---- Phase B: qT [768, 4096] from xqT ----
            with tc.tile_pool(name="psB", bufs=2, space="PSUM") as psb:
                pb = pa
                wq_sb = pb.tile([P, DT, D], f8, tag="wq_sb")
                wload(wq_sb, wq8T)
                for c in range(NQC):
                    xq8t = pb.tile([P, DT, QC], f8, tag="xq8t", bufs=4, name="xq8t")
                    for g in range(DT):
                        nc.sync.dma_start(
                            xq8t[:, g, :], xq8[g * P:(g + 1) * P, c * QC:(c + 1) * QC])
                    for go in range(DT):
                        ps = psb.tile([P, QC], f32, tag="pq")
                        for g in range(DT // 2):
                            nc.tensor.matmul(
                                ps[:], wq_sb[:, 2 * g:2 * g + 2, go * P:(go + 1) * P],
                                xq8t[:, 2 * g:2 * g + 2, :],
                                start=(g == 0), stop=(g == DT // 2 - 1),
                                perf_mode=DR)
                        nc.vector.tensor_scalar_add(
                            qT8[:, go, c * QC:(c + 1) * QC], ps[:], bQ(go))
            _ab_stack.close()

            # ---- Phase C: attention; per-qc deferred epilogue; chunked RS ----
            # Loads needed later emitted here so they overlap attention.
            wload(wc_sb, wcT, split=False)
            nc.sync.dma_start(bc_sb[:], bc[:])
            nc.gpsimd.partition_broadcast(bcb[:], bc_sb[:])

            with tc.tile_pool(name="pC", bufs=2) as pc, \
                 tc.tile_pool(name="pE", bufs=2) as pe, \
                 tc.tile_pool(name="psC", bufs=1, space="PSUM") as psc:
                f_view = f_dram[:].rearrange("a b -> (a b)").rearrange(
                    "(x c) -> x c", c=S)

                def emit_norm(b):
                    s_row = pe.tile([1, QC], bf16, tag="s_row", name="s_row")
                    nc.sync.dma_start(s_row[:], rs_out[b][FH:FH + 1, :])
                    frs = []
                    for r in range(FH // P):
                        fr = pe.tile([P, QC], bf16, tag="fr", bufs=3, name="fr")
                        nc.sync.dma_start(fr[:], rs_out[b][r * P:(r + 1) * P, :])
                        frs.append(fr)
                    s32 = pe.tile([1, QC], f32, tag="s32", name="s32")
                    nc.scalar.activation(s32[:], s_row[:], Copy)
                    rec = pe.tile([1, QC], f32, tag="rec", name="rec")
                    nc.vector.reciprocal_approx_fast(rec[:], s32[:])
                    rbc = pe.tile([P, QC], f32, tag="rbc", name="rbc")
                    nc.gpsimd.partition_broadcast(rbc[:], rec[:])
                    for r in range(FH // P):
                        fn = pe.tile([P, QC], bf16, tag="fn", bufs=3, name="fn")
                        nc.vector.tensor_mul(fn[:], frs[r][:], rbc[:])
                        nc.sync.dma_start(
                            f_view[r * P:(r + 1) * P, b * QC:(b + 1) * QC], fn[:])

                pend = {"epi": None, "norms": []}

                def emit_epilogue():
                    # deferred store half of the qc epilogue: bias-fold the
                    # finished ypsum into bf16 yTaug, then kick the RS.
                    blk, sbc, ypsum = pend["epi"]
                    pend["epi"] = None
                    yb = yTaug[blk]
                    nc.sync.dma_start(yb[FH:FH + 1, :], sbc[0:1, :])
                    nc.sync.dma_start(yb[2 * FH + 1:2 * FH + 2, :], sbc[0:1, :])
                    for e in range(DT):
                        yt_sb = pc.tile([P, QC], bf16, tag="yt_sb", bufs=3)
                        # (sums_bcast * bv[e]) + ypsum  — folds the value bias
                        nc.vector.scalar_tensor_tensor(
                            yt_sb[:], sbc[:], bV(e), ypsum[e][:], mult, add)
                        row = e * P if e < 3 else (FH + 1) + (e - 3) * P
                        nc.sync.dma_start(yb[row:row + P, :], yt_sb[:])
                    nc.gpsimd.collective_compute(
                        "ReduceScatter", mybir.AluOpType.add,
                        replica_groups=GROUPS,
                        ins=[yTaug[blk].opt()], outs=[rs_out[blk].opt()])
                    pend["norms"].append(blk)

                for qc in range(NQC):
                    sums_acc = pc.tile([P, QC], f32, tag="sums_acc")
                    nc.vector.memset(sums_acc[:], 0.0)
                    ypsum = None
                    a_tiles = {}
                    for kt in range(KT):
                        aps = psc.tile([P, QC], f32, tag="att", bufs=2)
                        for g in range(DT // 2):
                            # fp8 DoubleRow: each matmul contracts TWO 128-row
                            # feature subtiles (lhsT/rhs are [128, 2, free])
                            nc.tensor.matmul(
                                aps[:], kT8[:, 2 * g:2 * g + 2, kt * P:(kt + 1) * P],
                                qT8[:, 2 * g:2 * g + 2, qc * QC:(qc + 1) * QC],
                                start=(g == 0), stop=(g == DT // 2 - 1),
                                perf_mode=DR)
                        if kt == 0:
                            # previous qc's store half lands here: the vector
                            # chain it gates (stt x6) drains while this qc's QK
                            # matmuls keep the tensor engine busy.
                            if pend["epi"] is not None:
                                emit_epilogue()
                            ypsum = [psc.tile([P, QC], f32, name=f"y{e}", tag=f"y{e}", bufs=1)
                                     for e in range(DT)]
                        if kt >= AVD:
                            for e in range(DT):
                                nc.tensor.matmul(
                                    ypsum[e][:], v_sb[kt - AVD][:, e * P:(e + 1) * P],
                                    a_tiles[kt - AVD][:],
                                    start=(kt - AVD == 0), stop=False)
                        if kt == 4:
                            # normalize blocks whose RS was issued >= 3 qc ago
                            # (~96us; the RS pipeline incl. backlog is ~45us
                            # deep) — mid-loop, where the vector queue has
                            # slack. The last few blocks drain in the tail.
                            while pend["norms"] and pend["norms"][0] <= qc - 3:
                                emit_norm(pend["norms"].pop(0))
                        a_sb = pc.tile([P, QC], bf16, tag="a_sb", bufs=4)
                        a_tiles[kt] = a_sb
                        nc.scalar.activation(a_sb[:], aps[:], Exp, scale=SCALE)
                        nc.vector.tensor_add(sums_acc[:], sums_acc[:], a_sb[:])
                    sbc = None
                    for j, kt in enumerate(range(KT - AVD, KT)):
                        for e in range(DT):
                            nc.tensor.matmul(
                                ypsum[e][:], v_sb[kt][:, e * P:(e + 1) * P],
                                a_tiles[kt][:],
                                start=(kt == 0), stop=(kt == KT - 1))
                        if kt == KT - 2:
                            # sums half of the epilogue: cross-partition sum +
                            # broadcast via bf16 ones matmul, slotted between
                            # the AV tail groups so the cast has tensor cover.
                            sums_bf = pc.tile([P, QC], bf16, tag="sums_bf")
                            nc.vector.tensor_copy(sums_bf[:], sums_acc[:])
                            sp = psc.tile([P, QC], f32, tag="att", bufs=2)
                            nc.tensor.matmul(sp[:], ones_sb[:], sums_bf[:],
                                             start=True, stop=True)
                            sbc = pc.tile([P, QC], bf16, tag="sbc")
                            nc.vector.tensor_copy(sbc[:], sp[:])
                    pend["epi"] = (qc, sbc, ypsum)

                emit_epilogue()
                while pend["norms"]:
                    emit_norm(pend["norms"].pop(0))

            # ---- Phase F: out = y_perm @ Wc.T + bc ----
            with tc.tile_pool(name="pF", bufs=1) as pf, \
                 tc.tile_pool(name="psF", bufs=2, space="PSUM") as psf:
                # first transpose covers a single 128-row tile so phase F's
                # first matmul starts as soon as possible after the last norm;
                # the rest stream in behind it.
                tsplit = [(0, P), (P, 3 * P), (QC, QC), (2 * QC, QC), (3 * QC, QC)]
                fT4s = []
                for (row0, nrow) in tsplit:
                    fT4 = pf.tile([P, DT, nrow], bf16, name=f"fT4_{row0}", tag=f"fT4_{row0}")
                    nc.sync.dma_start_transpose(fT4[:], f_dram[row0:row0 + nrow, :])
                    fT4s.append(fT4)
                tmap = []
                for seg, (row0, nrow) in enumerate(tsplit):
                    for j in range(nrow // P):
                        tmap.append((seg, j * P))
                for t in range(SK // P):
                    seg, r0 = tmap[t]
                    fT = fT4s[seg]
                    po = psf.tile([P, QC], f32, tag="po", bufs=3)
                    po2 = psf.tile([P, D - QC], f32, tag="po2", bufs=3)
                    for gi in range(DT):
                        nc.tensor.matmul(po[:], fT[:, gi, r0:r0 + P], wc_sb[:, gi, 0:QC],
                                         start=(gi == 0), stop=(gi == DT - 1))
                        nc.tensor.matmul(po2[:], fT[:, gi, r0:r0 + P], wc_sb[:, gi, QC:D],
                                         start=(gi == 0), stop=(gi == DT - 1))
                    o_sb = pf.tile([P, D], f32, tag="o_sb", bufs=3)
                    nc.vector.tensor_add(o_sb[:, 0:QC], po[:], bcb[:, 0:QC])
                    nc.vector.tensor_add(o_sb[:, QC:D], po2[:], bcb[:, QC:D])
                    nc.sync.dma_start(out[t * P:(t + 1) * P, :], o_sb[:])

    return nc


def _get_nc():
    global _nc
    if _nc is None:
        _nc = _build_program()
        _nc.finalize()
    return _nc


def _prep_in_maps(x, Wq, bq, Wk, bk, Wv, bv, Wc, bc):
    F8 = ml_dtypes.float8_e4m3fn
    x = np.asarray(x, dtype=np.float32)
    wq8T = np.ascontiguousarray(np.asarray(Wq, np.float32).T).astype(F8)
    wk8T = np.ascontiguousarray(np.asarray(Wk, np.float32).T).astype(F8)
    wvT = np.ascontiguousarray(np.asarray(Wv, np.float32).T).astype(BF16)
    wcT = np.ascontiguousarray(np.asarray(Wc, np.float32).T).astype(BF16)
    bqkv = np.concatenate(
        [np.asarray(b, np.float32).reshape(DT, P).T for b in (bq, bk, bv)],
        axis=1).copy()
    bcc = np.asarray(bc, np.float32).reshape(1, D).copy()
    xTs = [np.ascontiguousarray(x[b].T) for b in range(B)]
    x8s = [xT.astype(F8) for xT in xTs]
    xbs = [xT.astype(BF16) for xT in xTs]
    in_maps = []
    for c in range(8):
        b, h = divmod(c, 2)
        in_maps.append({
            "xq8": x8s[b],
            "xkv8": np.ascontiguousarray(x8s[b][:, h * SK:(h + 1) * SK]),
            "xkvT": np.ascontiguousarray(xbs[b][:, h * SK:(h + 1) * SK]),
            "wq8T": wq8T, "wk8T": wk8T, "wvT": wvT, "wcT": wcT,
            "bqkv": bqkv, "bc": bcc,
        })
    return in_maps


def _assemble(results):
    out = np.empty((B, S, D), dtype=np.float32)
    for c in range(8):
        b, h = divmod(c, 2)
        out[b, h * SK:(h + 1) * SK, :] = results[c]["out"]
    return out


def run_on_hw(trace=False, **inputs):
    from concourse.bass_utils import run_bass_kernel_spmd
    nc = _get_nc()
    in_maps = _prep_in_maps(**inputs)
    res = run_bass_kernel_spmd(nc, in_maps, list(range(8)), trace=trace)
    return _assemble(res.results), res


def kernel(**inputs):
    out, _ = run_on_hw(trace=False, **inputs)
    return out


# revision 37
# speedup vs baseline: 1.0955x; 1.0108x over previous
"""Trainium2 Bass kernel for single-head full-dim attention (nn_CasualSelfAttention).

Reference math (B=4, S=4096, D=768, fp32):
    q = x @ Wq.T + bq ; k = x @ Wk.T + bk ; v = x @ Wv.T + bv
    att = softmax(q @ k.T * D**-0.5)        # no mask
    y = att @ v
    y = y.transpose(0,2,1).reshape(B,S,D)   # element permutation
    out = y @ Wc.T + bc

Sharding (8 cores): core c = 2*b + h handles batch b with ALL 4096 queries but
only its half of the keys/values (rows h*2048:(h+1)*2048). Each core produces a
partial unnormalized yT [768, 4096] (features x queries) plus partial softmax
sums, with the value bias folded in linearly (bv x partial_sums). A pairwise
ReduceScatter(add) in bf16, chunked per 512-query block, hands core h the
fully-reduced feature slice [384*h : 384*h+384] for all queries — exactly the
rows of y.T that the permutation maps to output rows [2048*h : 2048*h+2048].
After normalizing by the (also-reduced) sums, the flat buffer IS y_perm
row-major, and the final projection runs locally.

Perf notes vs the first working version:
  - Host pre-transposes x (xqT/xkvT) so no on-device DMA transposes are needed
    for the projections; biases ship as one packed [128,18] tensor (the 18
    separate 512B DMAs used to serialize ~12us of sync-engine time at startup).
  - The per-qc epilogue (sums reduce matmul in bf16, bias-fold, yTaug stores,
    RS issue) is emitted at kt==0 of the FOLLOWING qc, and the attention
    y-matmuls run at pipeline depth 2 (AV(kt-2) issues at kt), so the tensor
    engine has queued work while the vector engine drains the boundary chain.
  - Softmax-sum normalization of block b runs mid-loop (kt==10) of block b+1,
    using reciprocal_approx_fast, with the bf16->fp32 sum copy on the scalar
    engine — keeps the vector FIFO clear at qc boundaries.
  - Phase F uses 4 [512,768] DMA transposes instead of 16 [128,768] ones
    (the sync engine serializes ~2.4us per transpose issue).
"""

import numpy as np
import ml_dtypes

BF16 = ml_dtypes.bfloat16

B, S, D = 4, 4096, 768
SK = S // 2            # keys per core
P = 128
DT = D // P            # 6 feature tiles
KT = SK // P           # 16 key tiles
QC = 512               # query chunk width
NQC = S // QC          # 8 query chunks (= RS blocks)
FH = D // 2            # 384: feature rows per RS chunk
SCALE = float(D) ** -0.5
GROUPS = [[0, 1], [2, 3], [4, 5], [6, 7]]
AVD = 2                # AV pipeline depth: AV(kt-AVD) issues at kt

_nc = None


def _build_program():
    import concourse.bass as bass
    import concourse.mybir as mybir
    import concourse.tile as tile
    from concourse import bacc

    f32 = mybir.dt.float32
    bf16 = mybir.dt.bfloat16
    f8 = mybir.dt.float8e4
    DR = mybir.MatmulPerfMode.DoubleRow
    Exp = mybir.ActivationFunctionType.Exp
    Copy = mybir.ActivationFunctionType.Copy
    mult = mybir.AluOpType.mult
    add = mybir.AluOpType.add

    nc = bacc.Bacc(None, num_devices=8)

    xq8 = nc.declare_dram_parameter("xq8", [D, S], f8, isOutput=False)
    xkv8 = nc.declare_dram_parameter("xkv8", [D, SK], f8, isOutput=False)
    xkvT = nc.declare_dram_parameter("xkvT", [D, SK], bf16, isOutput=False)
    wq8T = nc.declare_dram_parameter("wq8T", [D, D], f8, isOutput=False)
    wk8T = nc.declare_dram_parameter("wk8T", [D, D], f8, isOutput=False)
    wvT = nc.declare_dram_parameter("wvT", [D, D], bf16, isOutput=False)
    wcT = nc.declare_dram_parameter("wcT", [D, D], bf16, isOutput=False)
    bqkv = nc.declare_dram_parameter("bqkv", [P, 3 * DT], f32, isOutput=False)
    bc = nc.declare_dram_parameter("bc", [1, D], f32, isOutput=False)
    out = nc.declare_dram_parameter("out", [SK, D], f32, isOutput=True)

    def wload(dst, src, split=True):
        # [768, 768] row-major -> [128, 6, 768] with logical row g*128+p.
        # split=True issues 6 independent per-g DMAs (parallel queues, lands
        # ~6x sooner); split=False is one strided DMA for non-urgent loads.
        if split:
            for g in range(DT):
                nc.sync.dma_start(dst[:, g, :], src[g * P:(g + 1) * P, :])
        else:
            nc.sync.dma_start(dst[:], src[:].rearrange("(g p) d -> p g d", p=P))

    with tile.TileContext(nc) as tc:
        with tc.tile_pool(name="persist", bufs=1) as pp, \
             tc.tile_pool(name="dram", bufs=1, space="DRAM") as dram:
            # Per block: rows 0:384 = feats 0:384, row 384 = partial sums,
            # rows 385:769 = feats 384:768, row 769 = partial sums.
            yTaug = [dram.tile([2 * (FH + 1), QC], bf16, name=f"yTaug{b}", tag=f"yTaug{b}")
                     for b in range(NQC)]
            rs_out = [dram.tile([FH + 1, QC], bf16, name=f"rs_out{b}", tag=f"rs_out{b}")
                      for b in range(NQC)]
            f_dram = dram.tile([SK, D], bf16)
            warm_cc_in = dram.tile([D, QC], bf16, name="warm_cc_in", tag="warm_cc_in")
            warm_cc_out = dram.tile([D // 2, QC], bf16, name="warm_cc_out", tag="warm_cc_out")

            # persistent SBUF: kT/qT (fp8, 3D so DoubleRow can pair feature
            # subtiles) + v activations + biases + output weights
            kT8 = pp.tile([P, DT, SK], f8, name="kT8", tag="kT8")
            qT8 = pp.tile([P, DT, S], f8, name="qT8", tag="qT8")
            v_sb = [pp.tile([P, D], bf16, name=f"v{t}", tag=f"v{t}") for t in range(KT)]
            bqkv_sb = pp.tile([P, 3 * DT], f32, tag="bqkv_sb")
            ones_sb = pp.tile([P, P], bf16, name="ones", tag="ones")
            wc_sb = pp.tile([P, DT, D], bf16, tag="wc_sb")
            bc_sb = pp.tile([1, D], f32, tag="bc_sb")
            bcb = pp.tile([P, D], f32, tag="bcb")

            def bQ(g):
                return bqkv_sb[:, g:g + 1]

            def bK(g):
                return bqkv_sb[:, DT + g:DT + g + 1]

            def bV(g):
                return bqkv_sb[:, 2 * DT + g:2 * DT + g + 1]

            # ---- Phase A: kT [768, 2048] and v [2048, 768] from xkvT ----
            import contextlib
            _ab_stack = contextlib.ExitStack()
            pa = _ab_stack.enter_context(tc.tile_pool(name="pA", bufs=1))
            with tc.tile_pool(name="psA", bufs=2, space="PSUM") as psa:
                wk_sb = pa.tile([P, DT, D], f8, tag="wk_sb")
                wload(wk_sb, wk8T)
                xk80 = pa.tile([P, DT, QC], f8, tag="xk8", bufs=2, name="xk8")
                for g in range(DT):
                    nc.sync.dma_start(xk80[:, g, :], xkv8[g * P:(g + 1) * P, 0:QC])
                nc.sync.dma_start(bqkv_sb[:], bqkv[:])
                xkT0 = pa.tile([P, DT, QC], bf16, tag="xkT", bufs=2, name="xkT")
                for g in range(DT):
                    nc.sync.dma_start(xkT0[:, g, :], xkvT[g * P:(g + 1) * P, 0:QC])
                wv_sb = pa.tile([P, DT, D], bf16, tag="wv_sb")
                wload(wv_sb, wvT)
                # Warm the PE while the first loads land: keeps the engine busy
                # through the HAM activity window so the projections start at
                # the full 2.4 GHz clock instead of the throttled 1.2 GHz.
                nc.vector.memset(ones_sb[:], 1.0)
                warm_ps = psa.tile([P, P], f32, tag="warm", bufs=1)
                for _ in range(65):
                    nc.tensor.matmul(warm_ps[:], ones_sb[:], ones_sb[:],
                                     start=True, stop=True)
                # Warm the collectives path too: the first collective of a run
                # pays ~20us of extra ncfw/ring setup — absorb it here, where
                # it overlaps the projections, instead of on block 0's RS.
                # Full-size so the descriptor rings are staged for the real
                # transfer size.
                nc.sync.dma_start(warm_cc_in[:], xkvT[:, 0:QC])
                nc.gpsimd.collective_compute(
                    "ReduceScatter", mybir.AluOpType.add,
                    replica_groups=GROUPS,
                    ins=[warm_cc_in.opt()], outs=[warm_cc_out.opt()])
                for c in range(SK // QC):
                    if c == 0:
                        xk8, xkT = xk80, xkT0
                    else:
                        xk8 = pa.tile([P, DT, QC], f8, tag="xk8", bufs=2, name="xk8")
                        for g in range(DT):
                            nc.sync.dma_start(
                                xk8[:, g, :], xkv8[g * P:(g + 1) * P, c * QC:(c + 1) * QC])
                        xkT = pa.tile([P, DT, QC], bf16, tag="xkT", bufs=2, name="xkT")
                        for g in range(DT):
                            nc.sync.dma_start(
                                xkT[:, g, :], xkvT[g * P:(g + 1) * P, c * QC:(c + 1) * QC])
                    for go in range(DT):
                        ps = psa.tile([P, QC], f32, tag="pk")
                        for g in range(DT // 2):
                            nc.tensor.matmul(
                                ps[:], wk_sb[:, 2 * g:2 * g + 2, go * P:(go + 1) * P],
                                xk8[:, 2 * g:2 * g + 2, :],
                                start=(g == 0), stop=(g == DT // 2 - 1),
                                perf_mode=DR)
                        nc.vector.tensor_scalar_add(
                            kT8[:, go, c * QC:(c + 1) * QC], ps[:], bK(go))
                    for tl in range(QC // P):
                        t = c * 4 + tl
                        for half in range(2):
                            ps = psa.tile([P, FH], f32, tag="pv", bufs=4)
                            for gi in range(DT):
                                nc.tensor.matmul(
                                    ps[:], xkT[:, gi, tl * P:(tl + 1) * P],
                                    wv_sb[:, gi, half * FH:(half + 1) * FH],
                                    start=(gi == 0), stop=(gi == DT - 1))
                            nc.vector.tensor_copy(v_sb[t][:, half * FH:(half + 1) * FH], ps[:])

            # ---- Phase B: qT [768, 4096] from xqT ----
            with tc.tile_pool(name="psB", bufs=2, space="PSUM") as psb:
                pb = pa
                wq_sb = pb.tile([P, DT, D], f8, tag="wq_sb")
                wload(wq_sb, wq8T)
                for c in range(NQC):
                    xq8t = pb.tile([P, DT, QC], f8, tag="xq8t", bufs=4, name="xq8t")
                    for g in range(DT):
                        nc.sync.dma_start(
                            xq8t[:, g, :], xq8[g * P:(g + 1) * P, c * QC:(c + 1) * QC])
                    for go in range(DT):
                        ps = psb.tile([P, QC], f32, tag="pq")
                        for g in range(DT // 2):
                            nc.tensor.matmul(
                                ps[:], wq_sb[:, 2 * g:2 * g + 2, go * P:(go + 1) * P],
                                xq8t[:, 2 * g:2 * g + 2, :],
                                start=(g == 0), stop=(g == DT // 2 - 1),
                                perf_mode=DR)
                        nc.vector.tensor_scalar_add(
                            qT8[:, go, c * QC:(c + 1) * QC], ps[:], bQ(go))
            _ab_stack.close()

            # ---- Phase C: attention; per-qc deferred epilogue; chunked RS ----
            # Loads needed later emitted here so they overlap attention.
            wload(wc_sb, wcT, split=False)
            nc.sync.dma_start(bc_sb[:], bc[:])
            nc.gpsimd.partition_broadcast(bcb[:], bc_sb[:])

            with tc.tile_pool(name="pC", bufs=2) as pc, \
                 tc.tile_pool(name="pE", bufs=2) as pe, \
                 tc.tile_pool(name="psC", bufs=1, space="PSUM") as psc:
                f_view = f_dram[:].rearrange("a b -> (a b)").rearrange(
                    "(x c) -> x c", c=S)

                def emit_norm(b):
                    s_row = pe.tile([1, QC], bf16, tag="s_row", name="s_row")
                    nc.sync.dma_start(s_row[:], rs_out[b][FH:FH + 1, :])
                    frs = []
                    for r in range(FH // P):
                        fr = pe.tile([P, QC], bf16, tag="fr", bufs=3, name="fr")
                        nc.sync.dma_start(fr[:], rs_out[b][r * P:(r + 1) * P, :])
                        frs.append(fr)
                    s32 = pe.tile([1, QC], f32, tag="s32", name="s32")
                    nc.scalar.activation(s32[:], s_row[:], Copy)
                    rec = pe.tile([1, QC], f32, tag="rec", name="rec")
                    nc.vector.reciprocal_approx_fast(rec[:], s32[:])
                    rbc = pe.tile([P, QC], f32, tag="rbc", name="rbc")
                    nc.gpsimd.partition_broadcast(rbc[:], rec[:])
                    for r in range(FH // P):
                        fn = pe.tile([P, QC], bf16, tag="fn", bufs=3, name="fn")
                        nc.vector.tensor_mul(fn[:], frs[r][:], rbc[:])
                        nc.sync.dma_start(
                            f_view[r * P:(r + 1) * P, b * QC:(b + 1) * QC], fn[:])

                pend = {"epi": None, "norms": []}

                def emit_epilogue():
                    # deferred store half of the qc epilogue: bias-fold the
                    # finished ypsum into bf16 yTaug, then kick the RS.
                    blk, sbc, ypsum = pend["epi"]
                    pend["epi"] = None
                    yb = yTaug[blk]
                    nc.sync.dma_start(yb[FH:FH + 1, :], sbc[0:1, :])
                    nc.sync.dma_start(yb[2 * FH + 1:2 * FH + 2, :], sbc[0:1, :])
                    for e in range(DT):
                        yt_sb = pc.tile([P, QC], bf16, tag="yt_sb", bufs=3)
                        # (sums_bcast * bv[e]) + ypsum  — folds the value bias
                        nc.vector.scalar_tensor_tensor(
                            yt_sb[:], sbc[:], bV(e), ypsum[e][:], mult, add)
                        row = e * P if e < 3 else (FH + 1) + (e - 3) * P
                        nc.sync.dma_start(yb[row:row + P, :], yt_sb[:])
                    nc.gpsimd.collective_compute(
                        "ReduceScatter", mybir.AluOpType.add,
                        replica_groups=GROUPS,
                        ins=[yTaug[blk].opt()], outs=[rs_out[blk].opt()])
                    pend["norms"].append(blk)

                for qc in range(NQC):
                    sums_acc = pc.tile([P, QC], f32, tag="sums_acc")
                    nc.vector.memset(sums_acc[:], 0.0)
                    ypsum = None
                    a_tiles = {}
                    for kt in range(KT):
                        aps = psc.tile([P, QC], f32, tag="att", bufs=2)
                        for g in range(DT // 2):
                            # fp8 DoubleRow: each matmul contracts TWO 128-row
                            # feature subtiles (lhsT/rhs are [128, 2, free])
                            nc.tensor.matmul(
                                aps[:], kT8[:, 2 * g:2 * g + 2, kt * P:(kt + 1) * P],
                                qT8[:, 2 * g:2 * g + 2, qc * QC:(qc + 1) * QC],
                                start=(g == 0), stop=(g == DT // 2 - 1),
                                perf_mode=DR)
                        if kt == 0:
                            # previous qc's store half lands here: the vector
                            # chain it gates (stt x6) drains while this qc's QK
                            # matmuls keep the tensor engine busy.
                            if pend["epi"] is not None:
                                emit_epilogue()
                            ypsum = [psc.tile([P, QC], f32, name=f"y{e}", tag=f"y{e}", bufs=1)
                                     for e in range(DT)]
                        if kt >= AVD:
                            for e in range(DT):
                                nc.tensor.matmul(
                                    ypsum[e][:], v_sb[kt - AVD][:, e * P:(e + 1) * P],
                                    a_tiles[kt - AVD][:],
                                    start=(kt - AVD == 0), stop=False)
                        if kt == 4:
                            # normalize blocks whose RS was issued >= 3 qc ago
                            # (~96us; the RS pipeline incl. backlog is ~45us
                            # deep) — mid-loop, where the vector queue has
                            # slack. The last few blocks drain in the tail.
                            while pend["norms"] and pend["norms"][0] <= qc - 3:
                                emit_norm(pend["norms"].pop(0))
                        a_sb = pc.tile([P, QC], bf16, tag="a_sb", bufs=4)
                        a_tiles[kt] = a_sb
                        nc.scalar.activation(a_sb[:], aps[:], Exp, scale=SCALE)
                        nc.vector.tensor_add(sums_acc[:], sums_acc[:], a_sb[:])
                    if qc < NQC - 1:
                        sbc = None
                        for j, kt in enumerate(range(KT - AVD, KT)):
                            for e in range(DT):
                                nc.tensor.matmul(
                                    ypsum[e][:], v_sb[kt][:, e * P:(e + 1) * P],
                                    a_tiles[kt][:],
                                    start=(kt == 0), stop=(kt == KT - 1))
                            if kt == KT - 2:
                                # sums half of the epilogue: cross-partition
                                # sum + broadcast via bf16 ones matmul, slotted
                                # between the AV tail groups so the cast has
                                # tensor cover.
                                sums_bf = pc.tile([P, QC], bf16, tag="sums_bf")
                                nc.vector.tensor_copy(sums_bf[:], sums_acc[:])
                                sp = psc.tile([P, QC], f32, tag="att", bufs=2)
                                nc.tensor.matmul(sp[:], ones_sb[:], sums_bf[:],
                                                 start=True, stop=True)
                                sbc = pc.tile([P, QC], bf16, tag="sbc")
                                nc.vector.tensor_copy(sbc[:], sp[:])
                        pend["epi"] = (qc, sbc, ypsum)
                    else:
                        # last qc: e-major tail so each feature chunk's store
                        # DMA departs as soon as ITS two AV groups finish —
                        # gets the final RS on the wire a few us earlier.
                        sums_bf = pc.tile([P, QC], bf16, tag="sums_bf")
                        nc.vector.tensor_copy(sums_bf[:], sums_acc[:])
                        sp = psc.tile([P, QC], f32, tag="att", bufs=2)
                        nc.tensor.matmul(sp[:], ones_sb[:], sums_bf[:],
                                         start=True, stop=True)
                        sbc = pc.tile([P, QC], bf16, tag="sbc")
                        nc.vector.tensor_copy(sbc[:], sp[:])
                        yb = yTaug[qc]
                        nc.sync.dma_start(yb[FH:FH + 1, :], sbc[0:1, :])
                        nc.sync.dma_start(yb[2 * FH + 1:2 * FH + 2, :], sbc[0:1, :])
                        for e in range(DT):
                            for kt in range(KT - AVD, KT):
                                nc.tensor.matmul(
                                    ypsum[e][:], v_sb[kt][:, e * P:(e + 1) * P],
                                    a_tiles[kt][:],
                                    start=False, stop=(kt == KT - 1))
                            yt_sb = pc.tile([P, QC], bf16, tag="yt_sb", bufs=3)
                            nc.vector.scalar_tensor_tensor(
                                yt_sb[:], sbc[:], bV(e), ypsum[e][:], mult, add)
                            row = e * P if e < 3 else (FH + 1) + (e - 3) * P
                            nc.sync.dma_start(yb[row:row + P, :], yt_sb[:])
                        nc.gpsimd.collective_compute(
                            "ReduceScatter", mybir.AluOpType.add,
                            replica_groups=GROUPS,
                            ins=[yTaug[qc].opt()], outs=[rs_out[qc].opt()])
                        pend["norms"].append(qc)

                while pend["norms"]:
                    emit_norm(pend["norms"].pop(0))

            # ---- Phase F: out = y_perm @ Wc.T + bc ----
            with tc.tile_pool(name="pF", bufs=1) as pf, \
                 tc.tile_pool(name="psF", bufs=2, space="PSUM") as psf:
                # first transpose covers a single 128-row tile so phase F's
                # first matmul starts as soon as possible after the last norm;
                # the rest stream in behind it.
                tsplit = [(0, P), (P, 3 * P), (QC, QC), (2 * QC, QC), (3 * QC, QC)]
                fT4s = []
                for (row0, nrow) in tsplit:
                    fT4 = pf.tile([P, DT, nrow], bf16, name=f"fT4_{row0}", tag=f"fT4_{row0}")
                    nc.sync.dma_start_transpose(fT4[:], f_dram[row0:row0 + nrow, :])
                    fT4s.append(fT4)
                tmap = []
                for seg, (row0, nrow) in enumerate(tsplit):
                    for j in range(nrow // P):
                        tmap.append((seg, j * P))
                for t in range(SK // P):
                    seg, r0 = tmap[t]
                    fT = fT4s[seg]
                    po = psf.tile([P, QC], f32, tag="po", bufs=4)
                    po2 = psf.tile([P, D - QC], f32, tag="po2", bufs=4)
                    for gi in range(DT):
                        nc.tensor.matmul(po[:], fT[:, gi, r0:r0 + P], wc_sb[:, gi, 0:QC],
                                         start=(gi == 0), stop=(gi == DT - 1))
                        nc.tensor.matmul(po2[:], fT[:, gi, r0:r0 + P], wc_sb[:, gi, QC:D],
                                         start=(gi == 0), stop=(gi == DT - 1))
                    o_sb = pf.tile([P, D], f32, tag="o_sb", bufs=4)
                    nc.vector.tensor_add(o_sb[:, 0:QC], po[:], bcb[:, 0:QC])
                    nc.vector.tensor_add(o_sb[:, QC:D], po2[:], bcb[:, QC:D])
                    nc.sync.dma_start(out[t * P:(t + 1) * P, :], o_sb[:])

    return nc


def _get_nc():
    global _nc
    if _nc is None:
        _nc = _build_program()
        _nc.finalize()
    return _nc


def _prep_in_maps(x, Wq, bq, Wk, bk, Wv, bv, Wc, bc):
    F8 = ml_dtypes.float8_e4m3fn
    x = np.asarray(x, dtype=np.float32)
    wq8T = np.ascontiguousarray(np.asarray(Wq, np.float32).T).astype(F8)
    wk8T = np.ascontiguousarray(np.asarray(Wk, np.float32).T).astype(F8)
    wvT = np.ascontiguousarray(np.asarray(Wv, np.float32).T).astype(BF16)
    wcT = np.ascontiguousarray(np.asarray(Wc, np.float32).T).astype(BF16)
    bqkv = np.concatenate(
        [np.asarray(b, np.float32).reshape(DT, P).T for b in (bq, bk, bv)],
        axis=1).copy()
    bcc = np.asarray(bc, np.float32).reshape(1, D).copy()
    xTs = [np.ascontiguousarray(x[b].T) for b in range(B)]
    x8s = [xT.astype(F8) for xT in xTs]
    xbs = [xT.astype(BF16) for xT in xTs]
    in_maps = []
    for c in range(8):
        b, h = divmod(c, 2)
        in_maps.append({
            "xq8": x8s[b],
            "xkv8": np.ascontiguousarray(x8s[b][:, h * SK:(h + 1) * SK]),
            "xkvT": np.ascontiguousarray(xbs[b][:, h * SK:(h + 1) * SK]),
            "wq8T": wq8T, "wk8T": wk8T, "wvT": wvT, "wcT": wcT,
            "bqkv": bqkv, "bc": bcc,
        })
    return in_maps


def _assemble(results):
    out = np.empty((B, S, D), dtype=np.float32)
    for c in range(8):
        b, h = divmod(c, 2)
        out[b, h * SK:(h + 1) * SK, :] = results[c]["out"]
    return out


def run_on_hw(trace=False, **inputs):
    from concourse.bass_utils import run_bass_kernel_spmd
    nc = _get_nc()
    in_maps = _prep_in_maps(**inputs)
    res = run_bass_kernel_spmd(nc, in_maps, list(range(8)), trace=trace)
    return _assemble(res.results), res


def kernel(**inputs):
    out, _ = run_on_hw(trace=False, **inputs)
    return out


# revision 38
# speedup vs baseline: 1.1124x; 1.0154x over previous
"""Trainium2 Bass kernel for single-head full-dim attention (nn_CasualSelfAttention).

Reference math (B=4, S=4096, D=768, fp32):
    q = x @ Wq.T + bq ; k = x @ Wk.T + bk ; v = x @ Wv.T + bv
    att = softmax(q @ k.T * D**-0.5)        # no mask
    y = att @ v
    y = y.transpose(0,2,1).reshape(B,S,D)   # element permutation
    out = y @ Wc.T + bc

Sharding (8 cores): core c = 2*b + h handles batch b with ALL 4096 queries but
only its half of the keys/values (rows h*2048:(h+1)*2048). Each core produces a
partial unnormalized yT [768, 4096] (features x queries) plus partial softmax
sums, with the value bias folded in linearly (bv x partial_sums). A pairwise
ReduceScatter(add) in bf16, chunked per 512-query block, hands core h the
fully-reduced feature slice [384*h : 384*h+384] for all queries — exactly the
rows of y.T that the permutation maps to output rows [2048*h : 2048*h+2048].
After normalizing by the (also-reduced) sums, the flat buffer IS y_perm
row-major, and the final projection runs locally.

Perf notes vs the first working version:
  - Host pre-transposes x (xqT/xkvT) so no on-device DMA transposes are needed
    for the projections; biases ship as one packed [128,18] tensor (the 18
    separate 512B DMAs used to serialize ~12us of sync-engine time at startup).
  - The per-qc epilogue (sums reduce matmul in bf16, bias-fold, yTaug stores,
    RS issue) is emitted at kt==0 of the FOLLOWING qc, and the attention
    y-matmuls run at pipeline depth 2 (AV(kt-2) issues at kt), so the tensor
    engine has queued work while the vector engine drains the boundary chain.
  - Softmax-sum normalization of block b runs mid-loop (kt==10) of block b+1,
    using reciprocal_approx_fast, with the bf16->fp32 sum copy on the scalar
    engine — keeps the vector FIFO clear at qc boundaries.
  - Phase F uses 4 [512,768] DMA transposes instead of 16 [128,768] ones
    (the sync engine serializes ~2.4us per transpose issue).
"""

import numpy as np
import ml_dtypes

BF16 = ml_dtypes.bfloat16

B, S, D = 4, 4096, 768
SK = S // 2            # keys per core
P = 128
DT = D // P            # 6 feature tiles
KT = SK // P           # 16 key tiles
QC = 512               # query chunk width
NQC = S // QC          # 8 query chunks (= RS blocks)
FH = D // 2            # 384: feature rows per RS chunk
SCALE = float(D) ** -0.5
GROUPS = [[0, 1], [2, 3], [4, 5], [6, 7]]
AVD = 3                # AV pipeline depth: AV(kt-AVD) issues at kt

_nc = None


def _build_program():
    import concourse.bass as bass
    import concourse.mybir as mybir
    import concourse.tile as tile
    from concourse import bacc

    f32 = mybir.dt.float32
    bf16 = mybir.dt.bfloat16
    f8 = mybir.dt.float8e4
    DR = mybir.MatmulPerfMode.DoubleRow
    Exp = mybir.ActivationFunctionType.Exp
    Copy = mybir.ActivationFunctionType.Copy
    mult = mybir.AluOpType.mult
    add = mybir.AluOpType.add

    nc = bacc.Bacc(None, num_devices=8)

    xq8 = nc.declare_dram_parameter("xq8", [D, S], f8, isOutput=False)
    xkv8 = nc.declare_dram_parameter("xkv8", [D, SK], f8, isOutput=False)
    xkvT = nc.declare_dram_parameter("xkvT", [D, SK], bf16, isOutput=False)
    wq8T = nc.declare_dram_parameter("wq8T", [D, D], f8, isOutput=False)
    wk8T = nc.declare_dram_parameter("wk8T", [D, D], f8, isOutput=False)
    wvT = nc.declare_dram_parameter("wvT", [D, D], bf16, isOutput=False)
    wcT = nc.declare_dram_parameter("wcT", [D, D], bf16, isOutput=False)
    bqkv = nc.declare_dram_parameter("bqkv", [P, 3 * DT], f32, isOutput=False)
    bc = nc.declare_dram_parameter("bc", [1, D], f32, isOutput=False)
    out = nc.declare_dram_parameter("out", [SK, D], f32, isOutput=True)

    def wload(dst, src, split=True):
        # [768, 768] row-major -> [128, 6, 768] with logical row g*128+p.
        # split=True issues 6 independent per-g DMAs (parallel queues, lands
        # ~6x sooner); split=False is one strided DMA for non-urgent loads.
        if split:
            for g in range(DT):
                nc.sync.dma_start(dst[:, g, :], src[g * P:(g + 1) * P, :])
        else:
            nc.sync.dma_start(dst[:], src[:].rearrange("(g p) d -> p g d", p=P))

    with tile.TileContext(nc) as tc:
        with tc.tile_pool(name="persist", bufs=1) as pp, \
             tc.tile_pool(name="dram", bufs=1, space="DRAM") as dram:
            # Per block: rows 0:384 = feats 0:384, row 384 = partial sums,
            # rows 385:769 = feats 384:768, row 769 = partial sums.
            yTaug = [dram.tile([2 * (FH + 1), QC], bf16, name=f"yTaug{b}", tag=f"yTaug{b}")
                     for b in range(NQC)]
            rs_out = [dram.tile([FH + 1, QC], bf16, name=f"rs_out{b}", tag=f"rs_out{b}")
                      for b in range(NQC)]
            f_dram = dram.tile([SK, D], bf16)
            warm_cc_in = dram.tile([D, QC], bf16, name="warm_cc_in", tag="warm_cc_in")
            warm_cc_out = dram.tile([D // 2, QC], bf16, name="warm_cc_out", tag="warm_cc_out")

            # persistent SBUF: kT/qT (fp8, 3D so DoubleRow can pair feature
            # subtiles) + v activations + biases + output weights
            kT8 = pp.tile([P, DT, SK], f8, name="kT8", tag="kT8")
            qT8 = pp.tile([P, DT, S], f8, name="qT8", tag="qT8")
            v_sb = [pp.tile([P, D], bf16, name=f"v{t}", tag=f"v{t}") for t in range(KT)]
            bqkv_sb = pp.tile([P, 3 * DT], f32, tag="bqkv_sb")
            ones_sb = pp.tile([P, P], bf16, name="ones", tag="ones")
            wc_sb = pp.tile([P, DT, D], bf16, tag="wc_sb")
            bc_sb = pp.tile([1, D], f32, tag="bc_sb")
            bcb = pp.tile([P, D], f32, tag="bcb")

            def bQ(g):
                return bqkv_sb[:, g:g + 1]

            def bK(g):
                return bqkv_sb[:, DT + g:DT + g + 1]

            def bV(g):
                return bqkv_sb[:, 2 * DT + g:2 * DT + g + 1]

            # ---- Phase A: kT [768, 2048] and v [2048, 768] from xkvT ----
            import contextlib
            _ab_stack = contextlib.ExitStack()
            pa = _ab_stack.enter_context(tc.tile_pool(name="pA", bufs=1))
            with tc.tile_pool(name="psA", bufs=2, space="PSUM") as psa:
                wk_sb = pa.tile([P, DT, D], f8, tag="wk_sb")
                wload(wk_sb, wk8T)
                xk80 = pa.tile([P, DT, QC], f8, tag="xk8", bufs=3, name="xk8")
                for g in range(DT):
                    nc.sync.dma_start(xk80[:, g, :], xkv8[g * P:(g + 1) * P, 0:QC])
                nc.sync.dma_start(bqkv_sb[:], bqkv[:])
                xkT0 = pa.tile([P, DT, QC], bf16, tag="xkT", bufs=3, name="xkT")
                for g in range(DT):
                    nc.sync.dma_start(xkT0[:, g, :], xkvT[g * P:(g + 1) * P, 0:QC])
                wv_sb = pa.tile([P, DT, D], bf16, tag="wv_sb")
                wload(wv_sb, wvT)
                # Warm the PE while the first loads land: keeps the engine busy
                # through the HAM activity window so the projections start at
                # the full 2.4 GHz clock instead of the throttled 1.2 GHz.
                nc.vector.memset(ones_sb[:], 1.0)
                warm_ps = psa.tile([P, P], f32, tag="warm", bufs=1)
                for _ in range(65):
                    nc.tensor.matmul(warm_ps[:], ones_sb[:], ones_sb[:],
                                     start=True, stop=True)
                # Warm the collectives path too: the first collective of a run
                # pays ~20us of extra ncfw/ring setup — absorb it here, where
                # it overlaps the projections, instead of on block 0's RS.
                # Full-size so the descriptor rings are staged for the real
                # transfer size.
                nc.sync.dma_start(warm_cc_in[:], xkvT[:, 0:QC])
                nc.gpsimd.collective_compute(
                    "ReduceScatter", mybir.AluOpType.add,
                    replica_groups=GROUPS,
                    ins=[warm_cc_in.opt()], outs=[warm_cc_out.opt()])
                for c in range(SK // QC):
                    if c == 0:
                        xk8, xkT = xk80, xkT0
                    else:
                        xk8 = pa.tile([P, DT, QC], f8, tag="xk8", bufs=3, name="xk8")
                        for g in range(DT):
                            nc.sync.dma_start(
                                xk8[:, g, :], xkv8[g * P:(g + 1) * P, c * QC:(c + 1) * QC])
                        xkT = pa.tile([P, DT, QC], bf16, tag="xkT", bufs=3, name="xkT")
                        for g in range(DT):
                            nc.sync.dma_start(
                                xkT[:, g, :], xkvT[g * P:(g + 1) * P, c * QC:(c + 1) * QC])
                    for go in range(DT):
                        ps = psa.tile([P, QC], f32, tag="pk")
                        for g in range(DT // 2):
                            nc.tensor.matmul(
                                ps[:], wk_sb[:, 2 * g:2 * g + 2, go * P:(go + 1) * P],
                                xk8[:, 2 * g:2 * g + 2, :],
                                start=(g == 0), stop=(g == DT // 2 - 1),
                                perf_mode=DR)
                        nc.vector.tensor_scalar_add(
                            kT8[:, go, c * QC:(c + 1) * QC], ps[:], bK(go))
                    for tl in range(QC // P):
                        t = c * 4 + tl
                        for half in range(2):
                            ps = psa.tile([P, FH], f32, tag="pv", bufs=4)
                            for gi in range(DT):
                                nc.tensor.matmul(
                                    ps[:], xkT[:, gi, tl * P:(tl + 1) * P],
                                    wv_sb[:, gi, half * FH:(half + 1) * FH],
                                    start=(gi == 0), stop=(gi == DT - 1))
                            nc.vector.tensor_copy(v_sb[t][:, half * FH:(half + 1) * FH], ps[:])

            # ---- Phase B: qT [768, 4096] from xqT ----
            with tc.tile_pool(name="psB", bufs=2, space="PSUM") as psb:
                pb = pa
                wq_sb = pb.tile([P, DT, D], f8, tag="wq_sb")
                wload(wq_sb, wq8T)
                for c in range(NQC):
                    xq8t = pb.tile([P, DT, QC], f8, tag="xq8t", bufs=4, name="xq8t")
                    for g in range(DT):
                        nc.sync.dma_start(
                            xq8t[:, g, :], xq8[g * P:(g + 1) * P, c * QC:(c + 1) * QC])
                    for go in range(DT):
                        ps = psb.tile([P, QC], f32, tag="pq")
                        for g in range(DT // 2):
                            nc.tensor.matmul(
                                ps[:], wq_sb[:, 2 * g:2 * g + 2, go * P:(go + 1) * P],
                                xq8t[:, 2 * g:2 * g + 2, :],
                                start=(g == 0), stop=(g == DT // 2 - 1),
                                perf_mode=DR)
                        nc.vector.tensor_scalar_add(
                            qT8[:, go, c * QC:(c + 1) * QC], ps[:], bQ(go))
            _ab_stack.close()

            # ---- Phase C: attention; per-qc deferred epilogue; chunked RS ----
            # Loads needed later emitted here so they overlap attention.
            wload(wc_sb, wcT, split=False)
            nc.sync.dma_start(bc_sb[:], bc[:])
            nc.gpsimd.partition_broadcast(bcb[:], bc_sb[:])

            with tc.tile_pool(name="pC", bufs=2) as pc, \
                 tc.tile_pool(name="pE", bufs=2) as pe, \
                 tc.tile_pool(name="psC", bufs=1, space="PSUM") as psc:
                f_view = f_dram[:].rearrange("a b -> (a b)").rearrange(
                    "(x c) -> x c", c=S)

                def emit_norm(b):
                    s_row = pe.tile([1, QC], bf16, tag="s_row", name="s_row")
                    nc.sync.dma_start(s_row[:], rs_out[b][FH:FH + 1, :])
                    frs = []
                    for r in range(FH // P):
                        fr = pe.tile([P, QC], bf16, tag="fr", bufs=3, name="fr")
                        nc.sync.dma_start(fr[:], rs_out[b][r * P:(r + 1) * P, :])
                        frs.append(fr)
                    s32 = pe.tile([1, QC], f32, tag="s32", name="s32")
                    nc.scalar.activation(s32[:], s_row[:], Copy)
                    rec = pe.tile([1, QC], f32, tag="rec", name="rec")
                    nc.vector.reciprocal_approx_fast(rec[:], s32[:])
                    rbc = pe.tile([P, QC], f32, tag="rbc", name="rbc")
                    nc.gpsimd.partition_broadcast(rbc[:], rec[:])
                    for r in range(FH // P):
                        fn = pe.tile([P, QC], bf16, tag="fn", bufs=3, name="fn")
                        nc.vector.tensor_mul(fn[:], frs[r][:], rbc[:])
                        nc.sync.dma_start(
                            f_view[r * P:(r + 1) * P, b * QC:(b + 1) * QC], fn[:])

                pend = {"epi": None, "norms": []}

                def emit_epilogue():
                    # deferred store half of the qc epilogue: bias-fold the
                    # finished ypsum into bf16 yTaug, then kick the RS.
                    blk, sbc, ypsum = pend["epi"]
                    pend["epi"] = None
                    yb = yTaug[blk]
                    nc.sync.dma_start(yb[FH:FH + 1, :], sbc[0:1, :])
                    nc.sync.dma_start(yb[2 * FH + 1:2 * FH + 2, :], sbc[0:1, :])
                    for e in range(DT):
                        yt_sb = pc.tile([P, QC], bf16, tag="yt_sb", bufs=3)
                        # (sums_bcast * bv[e]) + ypsum  — folds the value bias
                        nc.vector.scalar_tensor_tensor(
                            yt_sb[:], sbc[:], bV(e), ypsum[e][:], mult, add)
                        row = e * P if e < 3 else (FH + 1) + (e - 3) * P
                        nc.sync.dma_start(yb[row:row + P, :], yt_sb[:])
                    nc.gpsimd.collective_compute(
                        "ReduceScatter", mybir.AluOpType.add,
                        replica_groups=GROUPS,
                        ins=[yTaug[blk].opt()], outs=[rs_out[blk].opt()])
                    pend["norms"].append(blk)

                for qc in range(NQC):
                    sums_acc = pc.tile([P, QC], f32, tag="sums_acc")
                    nc.vector.memset(sums_acc[:], 0.0)
                    ypsum = None
                    a_tiles = {}
                    for kt in range(KT):
                        aps = psc.tile([P, QC], f32, tag="att", bufs=2)
                        for g in range(DT // 2):
                            # fp8 DoubleRow: each matmul contracts TWO 128-row
                            # feature subtiles (lhsT/rhs are [128, 2, free])
                            nc.tensor.matmul(
                                aps[:], kT8[:, 2 * g:2 * g + 2, kt * P:(kt + 1) * P],
                                qT8[:, 2 * g:2 * g + 2, qc * QC:(qc + 1) * QC],
                                start=(g == 0), stop=(g == DT // 2 - 1),
                                perf_mode=DR)
                        if kt == 0:
                            # previous qc's store half lands here: the vector
                            # chain it gates (stt x6) drains while this qc's QK
                            # matmuls keep the tensor engine busy.
                            if pend["epi"] is not None:
                                emit_epilogue()
                            ypsum = [psc.tile([P, QC], f32, name=f"y{e}", tag=f"y{e}", bufs=1)
                                     for e in range(DT)]
                        if kt >= AVD:
                            for e in range(DT):
                                nc.tensor.matmul(
                                    ypsum[e][:], v_sb[kt - AVD][:, e * P:(e + 1) * P],
                                    a_tiles[kt - AVD][:],
                                    start=(kt - AVD == 0), stop=False)
                        if kt == 4:
                            # normalize blocks whose RS was issued >= 3 qc ago
                            # (~96us; the RS pipeline incl. backlog is ~45us
                            # deep) — mid-loop, where the vector queue has
                            # slack. The last few blocks drain in the tail.
                            while pend["norms"] and pend["norms"][0] <= qc - 3:
                                emit_norm(pend["norms"].pop(0))
                        a_sb = pc.tile([P, QC], bf16, tag="a_sb", bufs=5)
                        a_tiles[kt] = a_sb
                        nc.scalar.activation(a_sb[:], aps[:], Exp, scale=SCALE)
                        nc.vector.tensor_add(sums_acc[:], sums_acc[:], a_sb[:])
                    if qc < NQC - 1:
                        sbc = None
                        for j, kt in enumerate(range(KT - AVD, KT)):
                            for e in range(DT):
                                nc.tensor.matmul(
                                    ypsum[e][:], v_sb[kt][:, e * P:(e + 1) * P],
                                    a_tiles[kt][:],
                                    start=(kt == 0), stop=(kt == KT - 1))
                            if kt == KT - 2:
                                # sums half of the epilogue: cross-partition
                                # sum + broadcast via bf16 ones matmul, slotted
                                # between the AV tail groups so the cast has
                                # tensor cover.
                                sums_bf = pc.tile([P, QC], bf16, tag="sums_bf")
                                nc.vector.tensor_copy(sums_bf[:], sums_acc[:])
                                sp = psc.tile([P, QC], f32, tag="att", bufs=2)
                                nc.tensor.matmul(sp[:], ones_sb[:], sums_bf[:],
                                                 start=True, stop=True)
                                sbc = pc.tile([P, QC], bf16, tag="sbc")
                                nc.vector.tensor_copy(sbc[:], sp[:])
                        pend["epi"] = (qc, sbc, ypsum)
                    else:
                        # last qc: e-major tail so each feature chunk's store
                        # DMA departs as soon as ITS two AV groups finish —
                        # gets the final RS on the wire a few us earlier.
                        sums_bf = pc.tile([P, QC], bf16, tag="sums_bf")
                        nc.vector.tensor_copy(sums_bf[:], sums_acc[:])
                        sp = psc.tile([P, QC], f32, tag="att", bufs=2)
                        nc.tensor.matmul(sp[:], ones_sb[:], sums_bf[:],
                                         start=True, stop=True)
                        sbc = pc.tile([P, QC], bf16, tag="sbc")
                        nc.vector.tensor_copy(sbc[:], sp[:])
                        yb = yTaug[qc]
                        nc.sync.dma_start(yb[FH:FH + 1, :], sbc[0:1, :])
                        nc.sync.dma_start(yb[2 * FH + 1:2 * FH + 2, :], sbc[0:1, :])
                        for e in range(DT):
                            for kt in range(KT - AVD, KT):
                                nc.tensor.matmul(
                                    ypsum[e][:], v_sb[kt][:, e * P:(e + 1) * P],
                                    a_tiles[kt][:],
                                    start=False, stop=(kt == KT - 1))
                            yt_sb = pc.tile([P, QC], bf16, tag="yt_sb", bufs=3)
                            nc.vector.scalar_tensor_tensor(
                                yt_sb[:], sbc[:], bV(e), ypsum[e][:], mult, add)
                            row = e * P if e < 3 else (FH + 1) + (e - 3) * P
                            nc.sync.dma_start(yb[row:row + P, :], yt_sb[:])
                        nc.gpsimd.collective_compute(
                            "ReduceScatter", mybir.AluOpType.add,
                            replica_groups=GROUPS,
                            ins=[yTaug[qc].opt()], outs=[rs_out[qc].opt()])
                        pend["norms"].append(qc)

                while pend["norms"]:
                    emit_norm(pend["norms"].pop(0))

            # ---- Phase F: out = y_perm @ Wc.T + bc ----
            with tc.tile_pool(name="pF", bufs=1) as pf, \
                 tc.tile_pool(name="psF", bufs=2, space="PSUM") as psf:
                # first transpose covers a single 128-row tile so phase F's
                # first matmul starts as soon as possible after the last norm;
                # the rest stream in behind it.
                tsplit = [(0, P), (P, 3 * P), (QC, QC), (2 * QC, QC), (3 * QC, QC)]
                fT4s = []
                for (row0, nrow) in tsplit:
                    fT4 = pf.tile([P, DT, nrow], bf16, name=f"fT4_{row0}", tag=f"fT4_{row0}")
                    nc.sync.dma_start_transpose(fT4[:], f_dram[row0:row0 + nrow, :])
                    fT4s.append(fT4)
                tmap = []
                for seg, (row0, nrow) in enumerate(tsplit):
                    for j in range(nrow // P):
                        tmap.append((seg, j * P))
                for t in range(SK // P):
                    seg, r0 = tmap[t]
                    fT = fT4s[seg]
                    po = psf.tile([P, QC], f32, tag="po", bufs=4)
                    po2 = psf.tile([P, D - QC], f32, tag="po2", bufs=4)
                    for gi in range(DT):
                        nc.tensor.matmul(po[:], fT[:, gi, r0:r0 + P], wc_sb[:, gi, 0:QC],
                                         start=(gi == 0), stop=(gi == DT - 1))
                        nc.tensor.matmul(po2[:], fT[:, gi, r0:r0 + P], wc_sb[:, gi, QC:D],
                                         start=(gi == 0), stop=(gi == DT - 1))
                    o_sb = pf.tile([P, D], f32, tag="o_sb", bufs=4)
                    nc.vector.tensor_add(o_sb[:, 0:QC], po[:], bcb[:, 0:QC])
                    nc.vector.tensor_add(o_sb[:, QC:D], po2[:], bcb[:, QC:D])
                    nc.sync.dma_start(out[t * P:(t + 1) * P, :], o_sb[:])

    return nc


def _get_nc():
    global _nc
    if _nc is None:
        _nc = _build_program()
        _nc.finalize()
    return _nc


def _prep_in_maps(x, Wq, bq, Wk, bk, Wv, bv, Wc, bc):
    F8 = ml_dtypes.float8_e4m3fn
    x = np.asarray(x, dtype=np.float32)
    wq8T = np.ascontiguousarray(np.asarray(Wq, np.float32).T).astype(F8)
    wk8T = np.ascontiguousarray(np.asarray(Wk, np.float32).T).astype(F8)
    wvT = np.ascontiguousarray(np.asarray(Wv, np.float32).T).astype(BF16)
    wcT = np.ascontiguousarray(np.asarray(Wc, np.float32).T).astype(BF16)
    bqkv = np.concatenate(
        [np.asarray(b, np.float32).reshape(DT, P).T for b in (bq, bk, bv)],
        axis=1).copy()
    bcc = np.asarray(bc, np.float32).reshape(1, D).copy()
    xTs = [np.ascontiguousarray(x[b].T) for b in range(B)]
    x8s = [xT.astype(F8) for xT in xTs]
    xbs = [xT.astype(BF16) for xT in xTs]
    in_maps = []
    for c in range(8):
        b, h = divmod(c, 2)
        in_maps.append({
            "xq8": x8s[b],
            "xkv8": np.ascontiguousarray(x8s[b][:, h * SK:(h + 1) * SK]),
            "xkvT": np.ascontiguousarray(xbs[b][:, h * SK:(h + 1) * SK]),
            "wq8T": wq8T, "wk8T": wk8T, "wvT": wvT, "wcT": wcT,
            "bqkv": bqkv, "bc": bcc,
        })
    return in_maps


def _assemble(results):
    out = np.empty((B, S, D), dtype=np.float32)
    for c in range(8):
        b, h = divmod(c, 2)
        out[b, h * SK:(h + 1) * SK, :] = results[c]["out"]
    return out


def run_on_hw(trace=False, **inputs):
    from concourse.bass_utils import run_bass_kernel_spmd
    nc = _get_nc()
    in_maps = _prep_in_maps(**inputs)
    res = run_bass_kernel_spmd(nc, in_maps, list(range(8)), trace=trace)
    return _assemble(res.results), res


def kernel(**inputs):
    out, _ = run_on_hw(trace=False, **inputs)
    return out
